# revision 1
# baseline (speedup 1.0000x reference)
"""Fused self-attention kernel for Trainium2 (8 NeuronCores, batch-parallel).

Computes, for X of shape (8, 4096, 64):
    out[b] = softmax(X[b] @ X[b].T, axis=-1) @ X[b]
with one batch per NeuronCore.

Algorithm per core (flash-style, everything stays on-chip):
  - XT (64, 4096) built from X via PE transposes; stored bf16 for QK^T.
  - X_ext (4096, 65) = [X | ones] stored float32r for the PV matmul.
  - For each 512-query block, in groups of 3 key-chunks (128 keys each):
      S^T chunk = XT[:, keys].T @ XT[:, queries]   (PSUM, f32)
      P^T = exp(S^T - 32)                          (ScalarE, PSUM -> SBUF f32r)
      Y^T_ext += X_ext[keys].T @ P^T               (PSUM accumulate, f32)
    Y^T_ext row 64 is the softmax denominator (ones column of X_ext).
  - Transpose Y^T_ext back (PE), divide by denominator (DVE), DMA out.

softmax(S) == softmax(S - 32) exactly; the global constant shift keeps
exp in fp32 range (max row dot product is ~110.6 for this input scale,
min row max is ~29, so exponents stay within [-3, 79]).
"""

import sys

for _p in ("/opt/trn_rl_repo",):
    if _p not in sys.path:
        sys.path.insert(0, _p)

from contextlib import ExitStack

import numpy as np

import concourse.bass as bass
import concourse.tile as tile
from concourse import bacc, mybir
from concourse import bass_utils
from concourse.masks import make_identity

B, S, D = 8, 4096, 64
SHIFT = 32.0
QB = 512  # queries per block
JC = 128  # keys per chunk
GROUP = 3  # key chunks per exp group (3 PSUM banks per group buffer)
N_JC = S // JC  # 32
N_QB = S // QB  # 8

F32 = mybir.dt.float32
F32R = mybir.dt.float32r
BF16 = mybir.dt.bfloat16


def _body(ctx: ExitStack, tc: tile.TileContext, out: bass.AP, x: bass.AP):
    nc = tc.nc

    singles = ctx.enter_context(tc.tile_pool(name="singles", bufs=1))
    xld_pool = ctx.enter_context(tc.tile_pool(name="xld", bufs=4))
    pt_pool = ctx.enter_context(tc.tile_pool(name="pt", bufs=3))
    ysb_pool = ctx.enter_context(tc.tile_pool(name="ysb", bufs=2))
    yout_pool = ctx.enter_context(tc.tile_pool(name="yout", bufs=4))
    st_ps = ctx.enter_context(tc.tile_pool(name="st", bufs=2, space="PSUM"))
    yacc_ps = ctx.enter_context(tc.tile_pool(name="yacc", bufs=1, space="PSUM"))
    ytr_ps = ctx.enter_context(tc.tile_pool(name="ytr", bufs=1, space="PSUM"))

    identity = singles.tile([128, 128], F32)
    make_identity(nc, identity)

    # exp bias (the constant shift) as a per-partition scalar
    bias = singles.tile([128, 1], F32)
    nc.vector.memset(bias, -SHIFT)

    xext = singles.tile([128, N_JC, D + 1], F32R)
    ones = singles.tile([128, N_JC], F32)
    nc.vector.memset(ones, 1.0)
    nc.vector.tensor_copy(xext[:, :, D], ones)

    xt = singles.tile([64, S], BF16)

    # Load X, build X_ext (f32r) and XT (bf16)
    for jc in range(N_JC):
        xld = xld_pool.tile([128, D], F32, tag="xld")
        nc.sync.dma_start(xld, x[jc * JC : (jc + 1) * JC, :])
        nc.vector.tensor_copy(xext[:, jc, 0:D], xld)
        xtr = ytr_ps.tile([64, 128], F32, tag="ytr")
        nc.tensor.transpose(xtr, xld, identity)
        nc.vector.tensor_copy(xt[:, jc * JC : (jc + 1) * JC], xtr)

    n_groups = (N_JC + GROUP - 1) // GROUP
    for qb in range(N_QB):
        yacc = yacc_ps.tile([D + 1, QB], F32, tag="yacc")
        q_sl = bass.ts(qb, QB)
        for g in range(n_groups):
            chunks = list(range(g * GROUP, min((g + 1) * GROUP, N_JC)))
            w = len(chunks)
            st = st_ps.tile([128, GROUP, QB], F32, tag="st")
            for ci, jc in enumerate(chunks):
                nc.tensor.matmul(
                    st[:, ci, :],
                    xt[:, jc * JC : (jc + 1) * JC],
                    xt[:, q_sl],
                    start=True,
                    stop=True,
                )
            pt = pt_pool.tile([128, GROUP, QB], F32R, tag="pt")
            nc.scalar.activation(
                pt[:, 0:w, :],
                st[:, 0:w, :],
                mybir.ActivationFunctionType.Exp,
                bias=bias,
                scale=1.0,
            )
            for ci, jc in enumerate(chunks):
                nc.tensor.matmul(
                    yacc[:, :],
                    xext[:, jc, :],
                    pt[:, ci, :],
                    start=(jc == 0),
                    stop=(jc == N_JC - 1),
                )

        ysb = ysb_pool.tile([D + 1, QB], F32, tag="ysb")
        nc.vector.tensor_copy(ysb, yacc)
        for c in range(QB // 128):
            ytr = ytr_ps.tile([128, D + 1], F32, tag="ytr")
            nc.tensor.transpose(
                ytr, ysb[:, c * 128 : (c + 1) * 128], identity[0 : D + 1, 0 : D + 1]
            )
            rinv = yout_pool.tile([128, 1], F32, tag="rinv")
            nc.vector.reciprocal(rinv, ytr[:, D : D + 1])
            yo = yout_pool.tile([128, D], F32, tag="yo")
            nc.vector.tensor_scalar_mul(yo, ytr[:, 0:D], rinv)
            nc.sync.dma_start(out[qb * QB + c * 128 : qb * QB + (c + 1) * 128, :], yo)


def build():
    nc = bacc.Bacc("TRN2", target_bir_lowering=False, debug=False, num_devices=B)
    x = nc.dram_tensor("X", (S, D), F32, kind="ExternalInput").ap()
    out = nc.dram_tensor("out", (S, D), F32, kind="ExternalOutput").ap()
    with tile.TileContext(nc) as tc:
        with ExitStack() as ctx:
            _body(ctx, tc, out, x)
    nc.compile()
    return nc


_NC = None


def run(X: np.ndarray, trace: bool = False, tmpdir: str | None = None):
    global _NC
    if _NC is None:
        _NC = build()
    X = np.asarray(X, dtype=np.float32)
    in_maps = [{"X": np.ascontiguousarray(X[b])} for b in range(B)]
    res = bass_utils.run_bass_kernel_spmd(
        _NC, in_maps, core_ids=list(range(B)), trace=trace, tmpdir=tmpdir
    )
    out = np.stack([res.results[b]["out"] for b in range(B)], axis=0).astype(np.float32)
    return out, res


def kernel(X: np.ndarray) -> np.ndarray:
    out, _ = run(X, trace=False)
    return out


# revision 2
# speedup vs baseline: 1.0507x; 1.0507x over previous
"""Fused self-attention kernel for Trainium2 (8 NeuronCores, batch-parallel).

Computes, for X of shape (8, 4096, 64):
    out[b] = softmax(X[b] @ X[b].T, axis=-1) @ X[b]
with one batch per NeuronCore.

Per-core algorithm (flash-style, fully on-chip):
  - XTdup (128, 4096) bf16: X^T replicated on partition halves 0-63 and
    64-127, built with paired PE transposes (col groups 0 / 64).
  - X_ext (4096, 65) = [X | ones] in float32r (PV stationary operand).
  - Per 512-query block, in groups of 4 key-chunks (128 keys each):
      S^T chunks via 2 row-packed (tile_position) K=64 bf16 matmuls per pair
      P^T = exp(S^T - 32) on ScalarE (one 2048-wide ACTIVATE per group)
      Y^T_ext(group) = X_ext^T @ P^T (f32r, PSUM-accumulated over the group)
      DVE accumulates group partials into SBUF.
    The ones column of X_ext makes row 64 the softmax denominator.
  - PE transposes Y^T_ext back, DVE divides by the denominator, DMA out.

softmax(S) == softmax(S - 32) exactly; the global shift keeps exp in
fp32 range (row maxima of S lie in [29, 111] for unit-normal X).

All PSUM traffic rotates through one pool of 2 x 4-bank slots: S^T group
tiles, PV partials, and transpose outputs borrow slots as they free up.
"""

import sys

for _p in ("/opt/trn_rl_repo",):
    if _p not in sys.path:
        sys.path.insert(0, _p)

from contextlib import ExitStack

import numpy as np

import concourse.bass as bass
import concourse.tile as tile
from concourse import bacc, mybir
from concourse import bass_utils
from concourse.masks import make_identity

B, S, D = 8, 4096, 64
SHIFT = 32.0
QB = 512  # queries per block
JC = 128  # keys per chunk
GROUP = 4  # key chunks per exp group
N_JC = S // JC  # 32
N_QB = S // QB  # 8
N_G = N_JC // GROUP  # 8

F32 = mybir.dt.float32
F32R = mybir.dt.float32r
BF16 = mybir.dt.bfloat16


def _body(ctx: ExitStack, tc: tile.TileContext, out: bass.AP, x: bass.AP):
    nc = tc.nc

    singles = ctx.enter_context(tc.tile_pool(name="singles", bufs=1))
    xld_pool = ctx.enter_context(tc.tile_pool(name="xld", bufs=4))
    pt_pool = ctx.enter_context(tc.tile_pool(name="pt", bufs=3))
    ysum_pool = ctx.enter_context(tc.tile_pool(name="ysum", bufs=2))
    yout_pool = ctx.enter_context(tc.tile_pool(name="yout", bufs=4))
    psum = ctx.enter_context(tc.tile_pool(name="ps", bufs=2, space="PSUM"))

    idbf = singles.tile([128, 128], BF16)
    make_identity(nc, idbf)
    idf32 = singles.tile([D + 1, D + 1], F32)
    make_identity(nc, idf32)

    bias = singles.tile([128, 1], F32)
    nc.vector.memset(bias, -SHIFT)

    xext = singles.tile([128, N_JC, D + 1], F32R)
    ones = singles.tile([128, N_JC], F32)
    nc.vector.memset(ones, 1.0)
    nc.vector.tensor_copy(xext[:, :, D], ones)

    xtdup = singles.tile([128, S], BF16)

    # Input phase: load X chunks, build X_ext (f32r) and XTdup (bf16).
    for jc in range(N_JC):
        xld = xld_pool.tile([128, D], F32, tag="xld")
        nc.sync.dma_start(xld, x[jc * JC : (jc + 1) * JC, :])
        nc.vector.tensor_copy(xext[:, jc, 0:D], xld)
        xbf = xld_pool.tile([128, D], BF16, tag="xbf")
        nc.vector.tensor_copy(xbf, xld)
        ptr = psum.tile([128, 128], BF16, tag="ps")
        nc.tensor.transpose(ptr[0:64, :], xbf, idbf, tile_position=(0, 0))
        nc.tensor.transpose(ptr[64:128, :], xbf, idbf, tile_position=(0, 64))
        nc.vector.tensor_copy(xtdup[:, jc * JC : (jc + 1) * JC], ptr)

    def emit_st(qb, g):
        """S^T matmuls for group g of query block qb: row-packed pairs."""
        st = psum.tile([128, GROUP, QB], F32, tag="ps")
        q0 = qb * QB
        for pair in range(GROUP // 2):
            jc0 = g * GROUP + 2 * pair
            jc1 = jc0 + 1
            nc.tensor.matmul(
                st[:, 2 * pair, :],
                xtdup[0:64, jc0 * JC : (jc0 + 1) * JC],
                xtdup[0:64, q0 : q0 + QB],
                start=True,
                stop=True,
                tile_position=(0, 0),
            )
            nc.tensor.matmul(
                st[:, 2 * pair + 1, :],
                xtdup[64:128, jc1 * JC : (jc1 + 1) * JC],
                xtdup[64:128, q0 : q0 + QB],
                start=True,
                stop=True,
                tile_position=(64, 0),
            )
        return st

    def emit_exp(st):
        pt = pt_pool.tile([128, GROUP, QB], F32R, tag="pt")
        nc.scalar.activation(
            pt[:, :, :],
            st[:, :, :],
            mybir.ActivationFunctionType.Exp,
            bias=bias,
            scale=1.0,
        )
        return pt

    def emit_pv(qb, g, pt, ysum):
        """PV for group g: accumulate into a borrowed PSUM slot, then DVE-add."""
        yp = psum.tile([D + 1, QB], F32, tag="ps")
        for ci in range(GROUP):
            jc = g * GROUP + ci
            nc.tensor.matmul(
                yp,
                xext[:, jc, :],
                pt[:, ci, :],
                start=(ci == 0),
                stop=(ci == GROUP - 1),
            )
        if g == 0:
            nc.vector.tensor_copy(ysum, yp)
        else:
            nc.vector.tensor_add(ysum, ysum, yp)

    for qb in range(N_QB):
        ysum = ysum_pool.tile([D + 1, QB], F32, tag="ysum")
        # Software-pipelined emission: keep the PE queue one S^T group
        # ahead of the ACT-gated PV matmuls.
        st_tiles = {0: emit_st(qb, 0)}
        pt_tiles = {}
        for g in range(N_G):
            if g + 1 < N_G:
                st_tiles[g + 1] = emit_st(qb, g + 1)
            pt_tiles[g] = emit_exp(st_tiles.pop(g))
            emit_pv(qb, g, pt_tiles.pop(g), ysum)

        # Transpose Y^T_ext back, normalize, store.
        ytr = psum.tile([128, GROUP, QB], F32, tag="ps")
        for c in range(QB // 128):
            nc.tensor.transpose(
                ytr[:, c, 0 : D + 1], ysum[:, c * 128 : (c + 1) * 128], idf32
            )
            rinv = yout_pool.tile([128, 1], F32, tag="rinv")
            nc.vector.reciprocal(rinv, ytr[:, c, D : D + 1])
            yo = yout_pool.tile([128, D], F32, tag="yo")
            nc.vector.tensor_scalar_mul(yo, ytr[:, c, 0:D], rinv)
            nc.sync.dma_start(out[qb * QB + c * 128 : qb * QB + (c + 1) * 128, :], yo)


def build():
    nc = bacc.Bacc("TRN2", target_bir_lowering=False, debug=False, num_devices=B)
    x = nc.dram_tensor("X", (S, D), F32, kind="ExternalInput").ap()
    out = nc.dram_tensor("out", (S, D), F32, kind="ExternalOutput").ap()
    with tile.TileContext(nc) as tc:
        with ExitStack() as ctx:
            _body(ctx, tc, out, x)
    nc.compile()
    return nc


_NC = None


def run(X: np.ndarray, trace: bool = False, tmpdir: str | None = None):
    global _NC
    if _NC is None:
        _NC = build()
    X = np.asarray(X, dtype=np.float32)
    in_maps = [{"X": np.ascontiguousarray(X[b])} for b in range(B)]
    res = bass_utils.run_bass_kernel_spmd(
        _NC, in_maps, core_ids=list(range(B)), trace=trace, tmpdir=tmpdir
    )
    out = np.stack([res.results[b]["out"] for b in range(B)], axis=0).astype(np.float32)
    return out, res


def kernel(X: np.ndarray) -> np.ndarray:
    out, _ = run(X, trace=False)
    return out


# revision 5
# speedup vs baseline: 1.4679x; 1.3971x over previous
"""Fused self-attention kernel for Trainium2 (8 NeuronCores, batch-parallel).

Computes, for X of shape (8, 4096, 64):
    out[b] = softmax(X[b] @ X[b].T, axis=-1) @ X[b]
with one batch per NeuronCore.

Per-core algorithm (flash-style, fully on-chip):
  - XTdup (128, 4096) bf16: X^T replicated on partition halves 0-63 and
    64-127 (built with bf16 DMA transposes), so S^T key-chunk matmuls
    (K=64) can be row-packed in pairs via tile_position (0,0)/(64,0) and
    run two-at-a-time on the PE array.
  - X_ext (4096, 65) = [X | ones] in float32r (PV stationary operand).
  - Per 512-query block, in groups of 3 key-chunks (128 keys each):
      S^T chunks = XT[keys].T @ XT[:, queries]     (bf16, PSUM 3 banks)
      P^T = exp(S^T - 32)                          (one 1536-wide ACTIVATE)
      Y^T_ext += X_ext[keys].T @ P^T               (f32r, PSUM-accumulated
                                                    across the whole block)
    The ones column of X_ext makes row 64 the softmax denominator.
  - PE transposes Y^T_ext back, DVE divides by the denominator, DMA out.

softmax(S) == softmax(S - 32) exactly; the global shift keeps exp within
fp32 range (row maxima of S lie in [29, 111] for unit-normal X).

PSUM budget: S^T double-buffer 2x3 banks + Y accumulator 1 + transpose 1.
"""

import sys

for _p in ("/opt/trn_rl_repo",):
    if _p not in sys.path:
        sys.path.insert(0, _p)

from contextlib import ExitStack

import numpy as np

import concourse.bass as bass
import concourse.tile as tile
from concourse import bacc, mybir
from concourse import bass_utils
from concourse.masks import make_identity

B, S, D = 8, 4096, 64
SHIFT = 32.0
QB = 512  # queries per block
JC = 128  # keys per chunk
GROUP = 3  # key chunks per exp group (PSUM banks per S^T buffer)
N_JC = S // JC  # 32
N_QB = S // QB  # 8

F32 = mybir.dt.float32
F32R = mybir.dt.float32r
BF16 = mybir.dt.bfloat16


def _body(ctx: ExitStack, tc: tile.TileContext, out: bass.AP, x: bass.AP):
    nc = tc.nc

    singles = ctx.enter_context(tc.tile_pool(name="singles", bufs=1))
    xld_pool = ctx.enter_context(tc.tile_pool(name="xld", bufs=4))
    pt_pool = ctx.enter_context(tc.tile_pool(name="pt", bufs=3))
    ysb_pool = ctx.enter_context(tc.tile_pool(name="ysb", bufs=2))
    yout_pool = ctx.enter_context(tc.tile_pool(name="yout", bufs=4))
    st_ps = ctx.enter_context(tc.tile_pool(name="st", bufs=2, space="PSUM"))
    yacc_ps = ctx.enter_context(tc.tile_pool(name="yacc", bufs=1, space="PSUM"))
    ytr_ps = ctx.enter_context(tc.tile_pool(name="ytr", bufs=1, space="PSUM"))

    idf32 = singles.tile([D + 1, D + 1], F32)
    make_identity(nc, idf32)
    idbf = singles.tile([128, 128], BF16)
    make_identity(nc, idbf)

    bias = singles.tile([128, 1], F32)
    nc.vector.memset(bias, -SHIFT)

    xext = singles.tile([128, N_JC, D + 1], F32R)
    ones = singles.tile([128, N_JC], F32)
    nc.vector.memset(ones, 1.0)
    nc.vector.tensor_copy(xext[:, :, D], ones)

    xtdup = singles.tile([128, S], BF16)

    # Input phase: load X chunks; build X_ext (f32r) and XTdup (bf16, both
    # partition halves) via paired PE transposes (col groups 0 / 64). The
    # transpose tiles rotate through the ytr/yacc PSUM slots, which are
    # otherwise idle until the first query block's epilogue.
    for jc in range(N_JC):
        xld = xld_pool.tile([128, D], F32, tag="xld")
        nc.sync.dma_start(xld, x[jc * JC : (jc + 1) * JC, :])
        nc.vector.tensor_copy(xext[:, jc, 0:D], xld)
        xbf = xld_pool.tile([128, D], BF16, tag="xbf")
        nc.vector.tensor_copy(xbf, xld)
        pool = ytr_ps if jc % 2 == 0 else yacc_ps
        tag = "ytr" if jc % 2 == 0 else "yacc"
        ptr = pool.tile([128, 128], BF16, tag=tag)
        nc.tensor.transpose(ptr[0:64, :], xbf, idbf, tile_position=(0, 0))
        nc.tensor.transpose(ptr[64:128, :], xbf, idbf, tile_position=(0, 64))
        nc.vector.tensor_copy(xtdup[:, jc * JC : (jc + 1) * JC], ptr)

    groups = []
    lo = 0
    while lo < N_JC:
        groups.append(list(range(lo, min(lo + GROUP, N_JC))))
        lo += GROUP
    n_g = len(groups)

    def emit_st(qb, g):
        """S^T matmuls for group g; chunk jc uses PE row half jc%2 so
        adjacent matmuls pack pairwise onto the array."""
        chunks = groups[g]
        st = st_ps.tile([128, GROUP, QB], F32, tag="st")
        q0 = qb * QB
        for ci, jc in enumerate(chunks):
            half = jc % 2
            rows = slice(64 * half, 64 * half + 64)
            nc.tensor.matmul(
                st[:, ci, :],
                xtdup[rows, jc * JC : (jc + 1) * JC],
                xtdup[rows, q0 : q0 + QB],
                start=True,
                stop=True,
                tile_position=(64 * half, 0),
            )
        return st

    def emit_exp(st, g):
        w = len(groups[g])
        pt = pt_pool.tile([128, GROUP, QB], F32R, tag="pt")
        nc.scalar.activation(
            pt[:, 0:w, :],
            st[:, 0:w, :],
            mybir.ActivationFunctionType.Exp,
            bias=bias,
            scale=1.0,
        )
        return pt

    def emit_pv(g, pt, yacc):
        for ci, jc in enumerate(groups[g]):
            nc.tensor.matmul(
                yacc,
                xext[:, jc, :],
                pt[:, ci, :],
                start=(jc == 0),
                stop=(jc == N_JC - 1),
            )

    for qb in range(N_QB):
        yacc = yacc_ps.tile([D + 1, QB], F32, tag="yacc")
        st_tiles = {g: emit_st(qb, g) for g in range(min(2, n_g))}
        for g in range(n_g):
            pt = emit_exp(st_tiles.pop(g), g)
            if g + 2 < n_g:
                st_tiles[g + 2] = emit_st(qb, g + 2)
            emit_pv(g, pt, yacc)

        ysb = ysb_pool.tile([D + 1, QB], F32, tag="ysb")
        nc.vector.tensor_copy(ysb, yacc)
        for c in range(QB // 128):
            ytr = ytr_ps.tile([128, D + 1], F32, tag="ytr")
            nc.tensor.transpose(ytr, ysb[:, c * 128 : (c + 1) * 128], idf32)
            rinv = yout_pool.tile([128, 1], F32, tag="rinv")
            nc.vector.reciprocal(rinv, ytr[:, D : D + 1])
            yo = yout_pool.tile([128, D], F32, tag="yo")
            nc.vector.tensor_scalar_mul(yo, ytr[:, 0:D], rinv)
            nc.sync.dma_start(out[qb * QB + c * 128 : qb * QB + (c + 1) * 128, :], yo)


def build():
    nc = bacc.Bacc("TRN2", target_bir_lowering=False, debug=False, num_devices=B)
    x = nc.dram_tensor("X", (S, D), F32, kind="ExternalInput").ap()
    out = nc.dram_tensor("out", (S, D), F32, kind="ExternalOutput").ap()
    with tile.TileContext(nc) as tc:
        with ExitStack() as ctx:
            _body(ctx, tc, out, x)
    nc.compile()
    return nc


_NC = None


def run(X: np.ndarray, trace: bool = False, tmpdir: str | None = None):
    global _NC
    if _NC is None:
        _NC = build()
    X = np.asarray(X, dtype=np.float32)
    in_maps = [{"X": np.ascontiguousarray(X[b])} for b in range(B)]
    res = bass_utils.run_bass_kernel_spmd(
        _NC, in_maps, core_ids=list(range(B)), trace=trace, tmpdir=tmpdir
    )
    out = np.stack([res.results[b]["out"] for b in range(B)], axis=0).astype(np.float32)
    return out, res


def kernel(X: np.ndarray) -> np.ndarray:
    out, _ = run(X, trace=False)
    return out


# revision 7
# speedup vs baseline: 1.6956x; 1.1551x over previous
"""Fused self-attention kernel for Trainium2 (8 NeuronCores, batch-parallel).

Computes, for X of shape (8, 4096, 64):
    out[b] = softmax(X[b] @ X[b].T, axis=-1) @ X[b]
with one batch per NeuronCore.

Per-core algorithm (flash-style, fully on-chip):
  - XTdup (128, 4096) bf16: X^T replicated on partition halves 0-63 and
    64-127 (built with paired PE transposes into col groups 0/64), so the
    K=64 S^T matmuls can be row-packed pairwise via tile_position and run
    two-at-a-time on the PE array.
  - X_ext (4096, 65) = [X | ones] in float32r (PV stationary operand).
  - Per 512-query block, in groups of 3 key-chunks (128 keys each):
      S^T chunks = XT[keys].T @ XT[:, queries]     (bf16, PSUM 3 banks)
      P^T = exp(S^T - 32)                          (one 1536-wide ACTIVATE)
      Y^T_ext += X_ext[keys].T @ P^T               (f32r, PSUM-accumulated
                                                    across the whole block)
    The ones column of X_ext makes row 64 the softmax denominator.
  - PE transposes Y^T_ext back, DVE divides by the denominator, DMA out.
  The group pipeline is flattened across query blocks: S^T emission runs
  two groups ahead of exp/PV so ScalarE (the bottleneck) never starves.

softmax(S) == softmax(S - 32) exactly; the global shift keeps exp within
fp32 range (row maxima of S lie in [29, 111] for unit-normal X).

PSUM budget: S^T double-buffer 2x3 banks + Y accumulator 1 + transpose 1.
"""

import sys

for _p in ("/opt/trn_rl_repo",):
    if _p not in sys.path:
        sys.path.insert(0, _p)

from contextlib import ExitStack

import numpy as np

import concourse.bass as bass
import concourse.tile as tile
from concourse import bacc, mybir
from concourse import bass_utils
from concourse.masks import make_identity

B, S, D = 8, 4096, 64
SHIFT = 32.0
QB = 512  # queries per block
JC = 128  # keys per chunk
GROUP = 3  # key chunks per exp group (PSUM banks per S^T buffer)
N_JC = S // JC  # 32
N_QB = S // QB  # 8

F32 = mybir.dt.float32
F32R = mybir.dt.float32r
BF16 = mybir.dt.bfloat16


def _body(ctx: ExitStack, tc: tile.TileContext, out: bass.AP, x: bass.AP):
    nc = tc.nc

    singles = ctx.enter_context(tc.tile_pool(name="singles", bufs=1))
    xld_pool = ctx.enter_context(tc.tile_pool(name="xld", bufs=3))
    pt_pool = ctx.enter_context(tc.tile_pool(name="pt", bufs=3))
    ysb_pool = ctx.enter_context(tc.tile_pool(name="ysb", bufs=2))
    yout_pool = ctx.enter_context(tc.tile_pool(name="yout", bufs=4))
    st_ps = ctx.enter_context(tc.tile_pool(name="st", bufs=2, space="PSUM"))
    yacc_ps = ctx.enter_context(tc.tile_pool(name="yacc", bufs=1, space="PSUM"))
    ytr_ps = ctx.enter_context(tc.tile_pool(name="ytr", bufs=1, space="PSUM"))

    idf32 = singles.tile([D + 1, D + 1], F32)
    make_identity(nc, idf32)
    idbf = singles.tile([128, 128], BF16)
    make_identity(nc, idbf)

    bias = singles.tile([128, 1], F32)
    nc.vector.memset(bias, -SHIFT)

    xext = singles.tile([128, N_JC, D + 1], F32R)
    ones = singles.tile([128, N_JC], F32)
    nc.vector.memset(ones, 1.0)
    nc.vector.tensor_copy(xext[:, :, D], ones)

    xtdup = singles.tile([128, S], BF16)

    # Input phase: 4 chunks per unit. DMA a (128, 4, 64) slab, convert to
    # f32r (X_ext) and bf16, then 4 paired PE transposes into one PSUM tile
    # and a single wide copy into XTdup. Units alternate between the ytr
    # and yacc PSUM slots (idle until the first epilogue / first PV).
    for u in range(N_JC // 4):
        xld = xld_pool.tile([128, 4, D], F32, tag="xld")
        src = x[u * 512 : (u + 1) * 512, :].rearrange("(c p) d -> p c d", p=128)
        nc.sync.dma_start(xld, src)
        nc.vector.tensor_copy(xext[:, 4 * u : 4 * u + 4, 0:D], xld)
        xbf = xld_pool.tile([128, 4, D], BF16, tag="xbf")
        nc.vector.tensor_copy(xbf, xld)
        pool, tag = (ytr_ps, "ytr") if u % 2 == 0 else (yacc_ps, "yacc")
        ptr = pool.tile([128, 4, 128], BF16, tag=tag)
        for c in range(4):
            nc.tensor.transpose(ptr[0:64, c, :], xbf[:, c, :], idbf, tile_position=(0, 0))
            nc.tensor.transpose(
                ptr[64:128, c, :], xbf[:, c, :], idbf, tile_position=(0, 64)
            )
        nc.vector.tensor_copy(
            xtdup[:, u * 512 : (u + 1) * 512].rearrange("p (c j) -> p c j", c=4), ptr
        )

    # Global flattened group schedule.
    groups = []  # (qb, [chunks])
    for qb in range(N_QB):
        lo = 0
        while lo < N_JC:
            groups.append((qb, list(range(lo, min(lo + GROUP, N_JC)))))
            lo += GROUP
    n_g = len(groups)

    def emit_st(i):
        qb, chunks = groups[i]
        st = st_ps.tile([128, GROUP, QB], F32, tag="st")
        q0 = qb * QB
        for ci, jc in enumerate(chunks):
            half = jc % 2
            rows = slice(64 * half, 64 * half + 64)
            nc.tensor.matmul(
                st[:, ci, :],
                xtdup[rows, jc * JC : (jc + 1) * JC],
                xtdup[rows, q0 : q0 + QB],
                start=True,
                stop=True,
                tile_position=(64 * half, 0),
            )
        return st

    def emit_exp(st, i):
        w = len(groups[i][1])
        pt = pt_pool.tile([128, GROUP, QB], F32R, tag="pt")
        nc.scalar.activation(
            pt[:, 0:w, :],
            st[:, 0:w, :],
            mybir.ActivationFunctionType.Exp,
            bias=bias,
            scale=1.0,
        )
        return pt

    def emit_pv(i, pt, yacc):
        for ci, jc in enumerate(groups[i][1]):
            nc.tensor.matmul(
                yacc,
                xext[:, jc, :],
                pt[:, ci, :],
                start=(jc == 0),
                stop=(jc == N_JC - 1),
            )

    def emit_epilogue(qb, yacc):
        ysb = ysb_pool.tile([D + 1, QB], F32, tag="ysb")
        nc.vector.tensor_copy(ysb, yacc)
        for c in range(QB // 128):
            ytr = ytr_ps.tile([128, D + 1], F32, tag="ytr")
            nc.tensor.transpose(ytr, ysb[:, c * 128 : (c + 1) * 128], idf32)
            rinv = yout_pool.tile([128, 1], F32, tag="rinv")
            nc.vector.reciprocal(rinv, ytr[:, D : D + 1])
            yo = yout_pool.tile([128, D], F32, tag="yo")
            nc.vector.tensor_scalar_mul(yo, ytr[:, 0:D], rinv)
            nc.sync.dma_start(out[qb * QB + c * 128 : qb * QB + (c + 1) * 128, :], yo)

    st_tiles = {0: emit_st(0), 1: emit_st(1)}
    yaccs = {}
    for i in range(n_g):
        qb, chunks = groups[i]
        if qb not in yaccs:
            yaccs[qb] = yacc_ps.tile([D + 1, QB], F32, tag="yacc", name="yacc")
        pt = emit_exp(st_tiles.pop(i), i)
        if i + 2 < n_g:
            st_tiles[i + 2] = emit_st(i + 2)
        emit_pv(i, pt, yaccs[qb])
        if chunks[-1] == N_JC - 1:
            emit_epilogue(qb, yaccs.pop(qb))


def build():
    nc = bacc.Bacc("TRN2", target_bir_lowering=False, debug=False, num_devices=B)
    x = nc.dram_tensor("X", (S, D), F32, kind="ExternalInput").ap()
    out = nc.dram_tensor("out", (S, D), F32, kind="ExternalOutput").ap()
    with tile.TileContext(nc) as tc:
        with ExitStack() as ctx:
            _body(ctx, tc, out, x)
    nc.compile()
    return nc


_NC = None


def run(X: np.ndarray, trace: bool = False, tmpdir: str | None = None):
    global _NC
    if _NC is None:
        _NC = build()
    X = np.asarray(X, dtype=np.float32)
    in_maps = [{"X": np.ascontiguousarray(X[b])} for b in range(B)]
    res = bass_utils.run_bass_kernel_spmd(
        _NC, in_maps, core_ids=list(range(B)), trace=trace, tmpdir=tmpdir
    )
    out = np.stack([res.results[b]["out"] for b in range(B)], axis=0).astype(np.float32)
    return out, res


def kernel(X: np.ndarray) -> np.ndarray:
    out, _ = run(X, trace=False)
    return out
